# revision 1
# baseline (speedup 1.0000x reference)
"""Trainium2 Bass kernel for a dense transformer block (self-attn + cross-attn + MLP).

Sharding: 8 cores = 4 batches x 2 query-halves. Host permutes tokens per core so
the core's 512 query tokens are local columns 0-511; causal zig-zag chunk
assignment (chunks {0,3} vs {1,2} of 256 tokens) balances attention work, and a
single uniform SPMD program runs on all cores (dummy kv slots masked via
per-core bias data).

Device: feature-major activations [C partitions x tokens]; residual stream and
LayerNorm statistics in fp32(r); all other matmul operands bf16 (fp32 PSUM
accumulation). Transposed-scores attention (scores^T[kv, q]) avoids transposes
entirely; softmax denominators come from a ones-column appended to V (M=65 AV
matmuls); exp without max-subtraction (scores are O(1) by construction);
normalization deferred to one PE-broadcast reciprocal multiply per head.
"""
import os
import numpy as np

B, T, Tp, C, H = 4, 1024, 256, 1024, 16
D = C // H           # 64
KC = C // 128        # 8
FF = 4 * C
FKC = FF // 128      # 32
TMY = 512
EPS = 1e-5
NEG = -120000.0      # pre-scale mask; *0.125 = -15000 -> exp == 0
NEGB = -15000.0      # post-scale mask (activation bias)
SCALE = 0.125

_CACHED = {}


def _build_nc(sim_mode=False, debug=False):
    v_hb0 = bool(int(os.environ.get("KV_HB0", "0")))      # scores reads at base 0
    v_gpb = bool(int(os.environ.get("KV_GPBCAST", "0")))  # R via gpsimd bcast
    v_e64 = bool(int(os.environ.get("KV_E64", "0")))      # R matmul with K=64
    v_m64 = bool(int(os.environ.get("KV_M64", "0")))      # AV M=64, no norm
    import concourse.mybir as mybir
    from concourse import bacc
    from concourse.tile import TileContext

    F32 = mybir.dt.float32
    F32R = mybir.dt.float32r
    BF16 = mybir.dt.bfloat16
    AF = mybir.ActivationFunctionType
    ALU = mybir.AluOpType
    GELU = AF.Tanh if sim_mode else AF.Gelu  # CoreSim has no Gelu

    nc = bacc.Bacc("TRN2", target_bir_lowering=False, debug=False)

    def din(name, shape, dt):
        return nc.declare_dram_parameter(name, list(shape), dt, isOutput=False)

    XT = din("XT", [C, T], F32R)
    XMY = din("XMY", [C, TMY], F32R)
    PKT = din("PKT", [C, Tp], BF16)
    WQKV = din("WQKV", [C, 3 * C], BF16)
    WO = din("WO", [C, C], BF16)
    WQ = din("WQ", [C, C], BF16)
    WKV = din("WKV", [C, 2 * C], BF16)
    WCO = din("WCO", [C, C], BF16)
    WFC = din("WFC", [C, FF], BF16)
    WFP = din("WFP", [FF, C], BF16)
    BQK = din("BQK", [128, 16], F32)
    BO8 = din("BO8", [128, KC], F32)
    BQ8 = din("BQ8", [128, KC], F32)
    BKC8 = din("BKC8", [128, KC], F32)
    BCO8 = din("BCO8", [128, KC], F32)
    BFC32 = din("BFC32", [128, FKC], F32)
    BFP8 = din("BFP8", [128, KC], F32)
    LN1W = din("LN1W", [128, KC], F32)
    LN1B = din("LN1B", [128, KC], F32)
    LNCW = din("LNCW", [128, KC], F32)
    LNCB = din("LNCB", [128, KC], F32)
    LN2W = din("LN2W", [128, KC], F32)
    LN2B = din("LN2B", [128, KC], F32)
    T1M = din("T1M", [128, 128], F32)
    M1M = din("M1M", [128, 256], F32)
    E2 = din("E2", [64, 128], BF16)
    ONL8 = din("ONL8", [128, 8], F32R)
    ON8B = din("ON8B", [8, 128], F32R)
    EPS8 = din("EPS8", [8, 1], F32)
    ONES16 = din("ONES16", [128, 16], BF16)
    KEEPBC = din("KEEPBC", [128, TMY], F32)
    VBBC = din("VBBC", [128, C], BF16)
    VCBBC = din("VCBBC", [128, C], BF16)
    PADS = din("PADS", [128, 16], F32)
    PADC = din("PADC", [128, 2], F32)
    OUT = nc.declare_dram_parameter("OUT", [C, TMY], F32, isOutput=True)
    if debug:
        BF16_ = BF16
        DBG = {}
        for name, shape, dt in [
            ("DA1", [C, T], BF16_), ("DQT", [C, TMY], BF16_),
            ("DKT", [C, T], BF16_), ("DVA", [128, KC * 16 * 65], BF16_),
            ("DYT", [C, TMY], BF16_), ("DXR1", [C, TMY], F32),
            ("DA2", [C, TMY], BF16_), ("DYCT", [C, TMY], BF16_),
            ("DXR2", [C, TMY], F32), ("DA3", [C, TMY], BF16_),
            ("DZRAW", [C, TMY], F32), ("DZP", [C, TMY], F32),
        ]:
            DBG[name] = nc.declare_dram_parameter(name, shape, dt, isOutput=True)

    # self-attn kv slots per q-chunk: (chunk, kind) kind: 0=full, 1=diag0, 2=diag1
    SLOTS = {0: [(4, 0), (5, 0), (0, 1), (1, 2)],
             1: [(0, 0), (1, 0), (4, 0), (5, 0), (6, 0), (7, 0), (2, 1), (3, 2)]}

    uid = [0]

    def nm(p):
        uid[0] += 1
        return f"{p}_{uid[0]}"

    with TileContext(nc) as tc:
        with tc.tile_pool(name="cst", bufs=1) as cst, \
             tc.tile_pool(name="act", bufs=1) as act, \
             tc.tile_pool(name="wp", bufs=2) as wp, \
             tc.tile_pool(name="tmp", bufs=3) as tmp, \
             tc.tile_pool(name="ex", bufs=6) as exp_pool, \
             tc.tile_pool(name="ps", bufs=4, space="PSUM") as ps:

            def dump(key, t, nch, w):
                if not debug:
                    return
                dst = DBG[key]
                for k in range(nch):
                    src = t[:, k, :w]
                    if src.dtype != dst.dtype:
                        src = src.bitcast(dst.dtype)
                    nc.sync.dma_start(out=dst[128 * k:128 * (k + 1), :], in_=src)

            def c_tile(name, dram, shape, dt):
                t = cst.tile(list(shape), dt, tag=name)
                nc.sync.dma_start(out=t, in_=dram[:, :])
                return t

            t1m = c_tile("t1m", T1M, [128, 128], F32)
            m1m = c_tile("m1m", M1M, [128, 256], F32)
            e2 = c_tile("e2", E2, [64, 128], BF16)
            onl8 = c_tile("onl8", ONL8, [128, 8], F32R)
            on8b = c_tile("on8b", ON8B, [8, 128], F32R)
            eps8 = c_tile("eps8", EPS8, [8, 1], F32)
            ones16 = c_tile("ones16", ONES16, [128, 16], BF16)
            keepbc = c_tile("keepbc", KEEPBC, [128, TMY], F32)
            vbbc = c_tile("vbbc", VBBC, [128, C], BF16)
            vcbbc = c_tile("vcbbc", VCBBC, [128, C], BF16)
            pads = c_tile("pads", PADS, [128, 16], F32)
            padc = c_tile("padc", PADC, [128, 2], F32)
            bqk = c_tile("bqk", BQK, [128, 16], F32)
            bo8 = c_tile("bo8", BO8, [128, KC], F32)
            bq8 = c_tile("bq8", BQ8, [128, KC], F32)
            bkc8 = c_tile("bkc8", BKC8, [128, KC], F32)
            bco8 = c_tile("bco8", BCO8, [128, KC], F32)
            bfc32 = c_tile("bfc32", BFC32, [128, FKC], F32)
            bfp8 = c_tile("bfp8", BFP8, [128, KC], F32)
            ln1w = c_tile("ln1w", LN1W, [128, KC], F32)
            ln1b = c_tile("ln1b", LN1B, [128, KC], F32)
            lncw = c_tile("lncw", LNCW, [128, KC], F32)
            lncb = c_tile("lncb", LNCB, [128, KC], F32)
            ln2w = c_tile("ln2w", LN2W, [128, KC], F32)
            ln2b = c_tile("ln2b", LN2B, [128, KC], F32)

            xmy = act.tile([128, KC, TMY], F32R, tag="res", bufs=2, name="xmy")
            for k in range(KC):
                nc.sync.dma_start(out=xmy[:, k, :], in_=XMY[128 * k:128 * (k + 1), :])
            pkT = act.tile([128, KC, Tp], BF16, tag="pkT", bufs=1)
            for k in range(KC):
                nc.sync.dma_start(out=pkT[:, k, :], in_=PKT[128 * k:128 * (k + 1), :])

            def layernorm(get_x, W, w_col, b_col, out_t):
                """get_x(k, nt) -> [128, ww] f32r AP. Writes out_t (bf16)."""
                ntile = (W + 511) // 512
                for nt in range(ntile):
                    w0 = nt * 512
                    ww = min(W, w0 + 512) - w0
                    xs = [get_x(k, nt) for k in range(KC)]
                    pm = ps.tile([8, 512], F32, tag="mm", name=nm("pm"))[:, :ww]
                    ps2 = ps.tile([8, 512], F32, tag="mm", name=nm("ps2"))[:, :ww]
                    for k in range(KC):
                        sq = tmp.tile([128, 512], F32R, tag="lnsq", bufs=2, name=nm("sq"))[:, :ww]
                        nc.scalar.activation(sq, xs[k], AF.Square)
                        nc.tensor.matmul(pm, onl8, xs[k],
                                         start=(k == 0), stop=(k == KC - 1))
                        nc.tensor.matmul(ps2, onl8, sq,
                                         start=(k == 0), stop=(k == KC - 1))
                    m8 = tmp.tile([8, 512], F32R, tag="lnm8", bufs=2, name=nm("m8"))[:, :ww]
                    nc.scalar.activation(m8, pm, AF.Copy, scale=1.0 / C)
                    m2 = tmp.tile([8, 512], F32, tag="lnf", bufs=3, name=nm("m2"))[:, :ww]
                    nc.scalar.activation(m2, pm, AF.Square, scale=1.0 / C)
                    var8 = tmp.tile([8, 512], F32, tag="lnf", bufs=3, name=nm("var8"))[:, :ww]
                    nc.vector.scalar_tensor_tensor(
                        out=var8, in0=ps2, scalar=1.0 / C, in1=m2,
                        op0=ALU.mult, op1=ALU.subtract)
                    sd8 = tmp.tile([8, 512], F32, tag="lnf", bufs=3, name=nm("sd8"))[:, :ww]
                    nc.scalar.activation(sd8, var8, AF.Sqrt, bias=eps8[:, :])
                    rs8 = tmp.tile([8, 512], F32R, tag="lnm8", bufs=2, name=nm("rs8"))[:, :ww]
                    with nc.allow_low_precision(reason="ln rstd f32r"):
                        nc.vector.reciprocal(rs8, sd8)
                    pmb = ps.tile([128, 512], F32, tag="lnb", bufs=2,
                                  name=nm("pmb"))[:, :ww]
                    nc.tensor.matmul(pmb, on8b, m8, start=True, stop=True)
                    prb = ps.tile([128, 512], F32, tag="lnb", bufs=2,
                                  name=nm("prb"))[:, :ww]
                    nc.tensor.matmul(prb, on8b, rs8, start=True, stop=True)
                    for k in range(KC):
                        t = tmp.tile([128, 512], F32, tag="t512", bufs=3, name=nm("lt"))[:, :ww]
                        nc.vector.tensor_sub(t, get_x(k, nt), pmb)
                        nc.vector.scalar_tensor_tensor(
                            out=out_t[:, k, w0:w0 + ww], in0=t,
                            scalar=w_col[:, k:k + 1], in1=prb,
                            op0=ALU.mult, op1=ALU.mult)
                        nc.vector.tensor_scalar_add(
                            out_t[:, k, w0:w0 + ww], out_t[:, k, w0:w0 + ww],
                            b_col[:, k:k + 1])

            def wslab(Wdram, col0, ncols, kc0=0, nk=KC):
                t = wp.tile([128, KC, 512], BF16, tag="wslab", name=nm("ws"))[:, :nk, :ncols]
                for k in range(nk):
                    nc.sync.dma_start(
                        out=t[:, k, :],
                        in_=Wdram[128 * (kc0 + k):128 * (kc0 + k + 1),
                                  col0:col0 + ncols])
                return t

            def proj_fm(Wdram, col0, nout, rhs_t, Wtok, copyback):
                for g0 in range(0, nout, 4):
                    gn = min(4, nout - g0)
                    slab = wslab(Wdram, col0 + 128 * g0, 128 * gn)
                    for ml in range(gn):
                        m = g0 + ml
                        for nt in range((Wtok + 511) // 512):
                            w0 = nt * 512
                            ww = min(Wtok, w0 + 512) - w0
                            p = ps.tile([128, 512], F32, tag="mm",
                                        name=nm("pj"))[:, :ww]
                            for k in range(KC):
                                nc.tensor.matmul(
                                    p, slab[:, k, 128 * ml:128 * (ml + 1)],
                                    rhs_t[:, k, w0:w0 + ww],
                                    start=(k == 0), stop=(k == KC - 1))
                            copyback(p, m, w0, ww)

            def attention(kTt, vaugt, q_t, nslots_fn, pad_t, pad_col_fn, out_t,
                          masked):
                """Generic (self/cross) attention. out_t [128, KC, TMY] bf16."""
                for qc in range(2):
                    slots = nslots_fn(qc)
                    for hp in range(8):
                        pys = []
                        for h in (2 * hp, 2 * hp + 1):
                            hb = 64 * (h % 2)
                            py = ps.tile([65, 256], F32, tag="acc", bufs=2,
                                         name=nm("py"))[:(64 if v_m64 else 65), :]
                            hbr = 0 if v_hb0 else hb
                            for si, (ck, kind) in enumerate(slots):
                                psc = ps.tile([128, 256], F32, tag="mm",
                                              name=nm("sc"))
                                nc.tensor.matmul(
                                    psc,
                                    kTt[hbr:hbr + 64, h // 2, 128 * ck:128 * (ck + 1)],
                                    q_t[hbr:hbr + 64, h // 2, 256 * qc:256 * (qc + 1)],
                                    start=True, stop=True)
                                if masked and kind == 1:
                                    nc.vector.tensor_add(
                                        psc[:, 0:128], psc[:, 0:128], t1m)
                                elif masked and kind == 2:
                                    nc.vector.tensor_add(psc, psc, m1m)
                                ex = exp_pool.tile([128, 256], BF16, tag="ex")
                                col = pad_col_fn(qc, ck)
                                nc.scalar.activation(
                                    ex, psc, AF.Exp,
                                    bias=pad_t[:, col:col + 1], scale=SCALE)
                                nc.tensor.matmul(
                                    py,
                                    vaugt[:, ck, 65 * h:65 * h + (64 if v_m64 else 65)],
                                    ex,
                                    start=(si == 0), stop=(si == len(slots) - 1))
                            pys.append(py)
                        if v_m64:
                            for j, h in enumerate((2 * hp, 2 * hp + 1)):
                                hb = 64 * (h % 2)
                                nc.vector.tensor_copy(
                                    out_t[hb:hb + 64, h // 2,
                                          256 * qc:256 * (qc + 1)],
                                    pys[j][0:64, :])
                            continue
                        rsb = tmp.tile([128, 256], F32, tag="rsb", bufs=2)
                        if v_gpb:
                            for j in range(2):
                                n1 = tmp.tile([1, 256], F32, tag="n1",
                                              bufs=2, name=nm("n1"))
                                nc.vector.tensor_scalar_add(
                                    n1, pys[j][64:65, :], 1e-30)
                                r1 = tmp.tile([1, 256], F32, tag="r1",
                                              bufs=2, name=nm("r1"))
                                nc.vector.reciprocal(r1, n1)
                                rsbj = tmp.tile([64, 256], F32, tag="rsbj",
                                                bufs=2, name=nm("rj"))
                                nc.gpsimd.partition_broadcast(
                                    rsbj, r1, channels=64)
                                nc.vector.tensor_copy(
                                    rsb[64 * j:64 * (j + 1), :], rsbj)
                        elif v_e64:
                            n64 = tmp.tile([64, 256], F32, tag="n64", bufs=2,
                                           name=nm("n64"))
                            nc.vector.memset(n64, 0.0)
                            nc.vector.tensor_copy(n64[0:1, :], pys[0][64:65, :])
                            nc.vector.tensor_copy(n64[32:33, :], pys[1][64:65, :])
                            nc.vector.tensor_scalar_add(n64, n64, 1e-30)
                            rec64 = tmp.tile([64, 256], BF16, tag="rec64",
                                             bufs=2, name=nm("rc64"))
                            with nc.allow_low_precision(reason="softmax denom bf16"):
                                nc.vector.reciprocal(rec64, n64)
                            pr = ps.tile([128, 256], F32, tag="mm", name=nm("pr"))
                            nc.tensor.matmul(pr, e2[0:64, :], rec64,
                                             start=True, stop=True)
                            nc.vector.tensor_copy(rsb, pr)
                        else:
                            n33 = tmp.tile([33, 256], F32, tag="n33", bufs=2,
                                           name=nm("n33"))
                            nc.vector.memset(n33, 0.0)
                            nc.vector.tensor_copy(n33[0:1, :], pys[0][64:65, :])
                            nc.vector.tensor_copy(n33[32:33, :], pys[1][64:65, :])
                            nc.vector.tensor_scalar_add(n33, n33, 1e-30)
                            rec33 = tmp.tile([33, 256], BF16, tag="rec33", bufs=2,
                                             name=nm("rc"))
                            with nc.allow_low_precision(reason="softmax denom bf16"):
                                nc.vector.reciprocal(rec33, n33)
                            pr = ps.tile([128, 256], F32, tag="mm", name=nm("pr"))
                            nc.tensor.matmul(pr, e2[0:33, :], rec33, start=True, stop=True)
                            nc.vector.tensor_copy(rsb, pr)
                        for j, h in enumerate((2 * hp, 2 * hp + 1)):
                            hb = 64 * (h % 2)
                            nc.vector.tensor_mul(
                                out_t[hb:hb + 64, h // 2, 256 * qc:256 * (qc + 1)],
                                pys[j][0:64, :], rsb[hb:hb + 64, :])

            # ================ phase 1: LN1 + QKV ================
            a1 = act.tile([128, KC, T], BF16, tag="big", bufs=2, name="a1")

            def get_x1_fresh(k, nt):
                t = tmp.tile([128, 512], F32R, tag="xs", bufs=2)
                nc.sync.dma_start(
                    out=t, in_=XT[128 * k:128 * (k + 1), 512 * nt:512 * (nt + 1)])
                return t

            layernorm(get_x1_fresh, T, ln1w, ln1b, a1)
            dump("DA1", a1, KC, T)

            qT = act.tile([128, KC, TMY], BF16, tag="bb", bufs=2, name="qT")

            def cb_q(p, m, w0, ww):
                nc.vector.tensor_scalar_add(qT[:, m, w0:w0 + ww], p, bqk[:, m:m + 1])

            proj_fm(WQKV, 0, KC, a1, TMY, cb_q)
            dump("DQT", qT, KC, TMY)

            kT = act.tile([128, KC, T], BF16, tag="big", bufs=2, name="kT")

            def cb_k(p, m, w0, ww):
                nc.vector.tensor_scalar_add(kT[:, m, w0:w0 + ww], p,
                                            bqk[:, 8 + m:9 + m])

            proj_fm(WQKV, C, KC, a1, T, cb_k)
            dump("DKT", kT, KC, T)

            vaug = act.tile([128, KC, 16 * 65], BF16, tag="vaug", bufs=1)
            for t8 in range(KC):
                nc.sync.dma_start(
                    out=vaug.rearrange("p c (h q) -> p c h q", q=65)[:, t8, :, 64:65],
                    in_=ONES16.rearrange("p (h q) -> p h q", q=1))
            for g in range(2):
                slab = wslab(WQKV, 2 * C + 512 * g, 512)
                for t8 in range(KC):
                    p = ps.tile([128, 512], F32, tag="mm", name=nm("v"))
                    for k in range(KC):
                        nc.tensor.matmul(p, a1[:, k, 128 * t8:128 * (t8 + 1)],
                                         slab[:, k, :],
                                         start=(k == 0), stop=(k == KC - 1))
                    nc.vector.tensor_add(
                        vaug.rearrange("p c (h q) -> p c h q", q=65)[
                            :, t8, 8 * g:8 * (g + 1), 0:64],
                        p.rearrange("p (h q) -> p h q", q=64),
                        vbbc.rearrange("p (h q) -> p h q", q=64)[
                            :, 8 * g:8 * (g + 1), :])

            if debug:
                for k in range(KC):
                    nc.sync.dma_start(
                        out=DBG["DVA"][:, 16 * 65 * k:16 * 65 * (k + 1)],
                        in_=vaug[:, k, :])
            # ================ phase 2: self-attention ================
            yT = act.tile([128, KC, TMY], BF16, tag="bb", bufs=2, name="yT")
            attention(kT, vaug, qT, lambda qc: SLOTS[qc], pads,
                      lambda qc, ck: 8 * qc + ck, yT, masked=True)
            dump("DYT", yT, KC, TMY)

            # ================ phase 3: out-proj + residual ================
            xr1 = act.tile([128, KC, TMY], F32R, tag="res", bufs=2, name="xr1")

            def cb_z1(p, m, w0, ww):
                z = tmp.tile([128, 512], F32, tag="t512", bufs=3, name=nm("z"))[:, :ww]
                nc.vector.tensor_scalar_add(z, p, bo8[:, m:m + 1])
                zk = tmp.tile([128, 512], F32, tag="t512", bufs=3, name=nm("zk"))[:, :ww]
                nc.vector.tensor_mul(zk, z, keepbc[:, w0:w0 + ww])
                nc.vector.tensor_add(xr1[:, m, w0:w0 + ww], zk, xmy[:, m, w0:w0 + ww])

            proj_fm(WO, 0, KC, yT, TMY, cb_z1)
            dump("DXR1", xr1, KC, TMY)

            # ================ phase 4: cross-attention ================
            a2 = act.tile([128, KC, TMY], BF16, tag="bb", bufs=2, name="a2")
            layernorm(lambda k, nt: xr1[:, k, 512 * nt:512 * (nt + 1)],
                      TMY, lncw, lncb, a2)
            dump("DA2", a2, KC, TMY)

            qcT = act.tile([128, KC, TMY], BF16, tag="bb", bufs=2, name="qcT")

            def cb_qc(p, m, w0, ww):
                nc.vector.tensor_scalar_add(qcT[:, m, w0:w0 + ww], p, bq8[:, m:m + 1])

            proj_fm(WQ, 0, KC, a2, TMY, cb_qc)

            kcT = act.tile([128, KC, Tp], BF16, tag="kcT", bufs=1)

            def cb_kc(p, m, w0, ww):
                nc.vector.tensor_scalar_add(kcT[:, m, w0:w0 + ww], p,
                                            bkc8[:, m:m + 1])

            proj_fm(WKV, 0, KC, pkT, Tp, cb_kc)

            vcaug = act.tile([128, 2, 16 * 65], BF16, tag="vcaug", bufs=1)
            for t2 in range(2):
                nc.sync.dma_start(
                    out=vcaug.rearrange("p c (h q) -> p c h q", q=65)[:, t2, :, 64:65],
                    in_=ONES16.rearrange("p (h q) -> p h q", q=1))
            for g in range(2):
                slab = wslab(WKV, C + 512 * g, 512)
                for t2 in range(2):
                    p = ps.tile([128, 512], F32, tag="mm", name=nm("vc"))
                    for k in range(KC):
                        nc.tensor.matmul(p, pkT[:, k, 128 * t2:128 * (t2 + 1)],
                                         slab[:, k, :],
                                         start=(k == 0), stop=(k == KC - 1))
                    nc.vector.tensor_add(
                        vcaug.rearrange("p c (h q) -> p c h q", q=65)[
                            :, t2, 8 * g:8 * (g + 1), 0:64],
                        p.rearrange("p (h q) -> p h q", q=64),
                        vcbbc.rearrange("p (h q) -> p h q", q=64)[
                            :, 8 * g:8 * (g + 1), :])

            ycT = act.tile([128, KC, TMY], BF16, tag="bb", bufs=2, name="ycT")
            attention(kcT, vcaug, qcT, lambda qc: [(0, 0), (1, 0)], padc,
                      lambda qc, ck: ck, ycT, masked=False)
            dump("DYCT", ycT, KC, TMY)

            xr2 = act.tile([128, KC, TMY], F32R, tag="res", bufs=2, name="xr2")

            def cb_z2(p, m, w0, ww):
                z = tmp.tile([128, 512], F32, tag="t512", bufs=3, name=nm("z"))[:, :ww]
                nc.vector.tensor_scalar_add(z, p, bco8[:, m:m + 1])
                zk = tmp.tile([128, 512], F32, tag="t512", bufs=3, name=nm("zk"))[:, :ww]
                nc.vector.tensor_mul(zk, z, keepbc[:, w0:w0 + ww])
                nc.vector.tensor_add(xr2[:, m, w0:w0 + ww], zk, xr1[:, m, w0:w0 + ww])

            proj_fm(WCO, 0, KC, ycT, TMY, cb_z2)
            dump("DXR2", xr2, KC, TMY)

            # ================ phase 5: MLP ================
            a3 = act.tile([128, KC, TMY], BF16, tag="bb", bufs=2, name="a3")
            layernorm(lambda k, nt: xr2[:, k, 512 * nt:512 * (nt + 1)],
                      TMY, ln2w, ln2b, a3)
            dump("DA3", a3, KC, TMY)

            hT = act.tile([128, FKC, TMY], BF16, tag="hT", bufs=1)

            def cb_h(p, m, w0, ww):
                nc.scalar.activation(hT[:, m, w0:w0 + ww], p, GELU,
                                     bias=bfc32[:, m:m + 1])

            proj_fm(WFC, 0, FKC, a3, TMY, cb_h)

            outT = act.tile([128, KC, TMY], F32, tag="res", bufs=2, name="outT")
            for g in range(4):  # out-col groups of 256 (2 chunks)
                pouts = [ps.tile([128, 512], F32, tag="acc", bufs=2,
                                 name=nm("fp")) for _ in range(2)]
                for ksb in range(4):
                    slab = wslab(WFP, 256 * g, 256, kc0=8 * ksb, nk=8)
                    for i in range(2):
                        m = 2 * g + i
                        for k in range(8):
                            nc.tensor.matmul(
                                pouts[i], slab[:, k, 128 * i:128 * (i + 1)],
                                hT[:, 8 * ksb + k, :],
                                start=(ksb == 0 and k == 0),
                                stop=(ksb == 3 and k == 7))
                for i in range(2):
                    m = 2 * g + i
                    zf = tmp.tile([128, 512], F32, tag="t512", bufs=3,
                                  name=nm("zf"))
                    nc.vector.tensor_scalar_add(zf, pouts[i], bfp8[:, m:m + 1])
                    nc.vector.tensor_add(outT[:, m, :], zf, xr2[:, m, :])
            for m in range(KC):
                nc.sync.dma_start(out=OUT[128 * m:128 * (m + 1), :],
                                  in_=outT[:, m, :])


    nc.compile()
    return nc


def _host_prepare(core, inputs):
    import ml_dtypes
    BF = ml_dtypes.bfloat16
    b, half = core // 2, core % 2
    chunks = [0, 3, 1, 2] if half == 0 else [1, 2, 0, 3]
    idx = np.concatenate([np.arange(256 * c, 256 * (c + 1)) for c in chunks])

    x = np.asarray(inputs["x"], np.float32)
    pad = np.asarray(inputs["padding_mask"], bool)
    pocket = np.asarray(inputs["pocket_context"], np.float32)

    xT = np.ascontiguousarray(x[b][idx].T)
    pkT = np.ascontiguousarray(pocket[b].T).astype(BF)
    pad_loc = pad[b][idx]

    pads = np.zeros((128, 16), np.float32)
    attend = {0: {0, 1} | ({4, 5} if half == 1 else set()),
              1: {0, 1, 2, 3, 4, 5} | ({6, 7} if half == 0 else set())}
    for qc in range(2):
        for ck in range(8):
            col = 8 * qc + ck
            if ck not in attend[qc]:
                pads[:, col] = NEGB
            else:
                pads[:, col] = np.where(pad_loc[128 * ck:128 * (ck + 1)], NEGB, 0.0)
    pmask = np.abs(pocket[b]).sum(-1) == 0
    padc = np.zeros((128, 2), np.float32)
    for ck in range(2):
        padc[:, ck] = np.where(pmask[128 * ck:128 * (ck + 1)], NEGB, 0.0)
    keep = np.where(pad_loc[:TMY], 0.0, 1.0).astype(np.float32)

    t1 = np.where(np.arange(128)[None, :] >= np.arange(128)[:, None], 0.0,
                  NEG).astype(np.float32)
    m1 = np.concatenate([np.full((128, 128), NEG, np.float32), t1], axis=1)
    e2 = np.zeros((64, 128), np.float32)
    e2[0, 0:64] = 1.0
    e2[32, 64:128] = 1.0

    def pp(v):
        return np.ascontiguousarray(np.asarray(v, np.float32).reshape(-1, 128).T)

    def w(name):
        return np.asarray(inputs[name], np.float32).astype(BF)

    bqkv = np.asarray(inputs["bqkv"], np.float32)
    bkv = np.asarray(inputs["bkv"], np.float32)
    return {
        "XT": xT, "XMY": np.ascontiguousarray(xT[:, :TMY]), "PKT": pkT,
        "WQKV": w("Wqkv"), "WO": w("Wo"), "WQ": w("Wq"), "WKV": w("Wkv"),
        "WCO": w("Wco"), "WFC": w("Wfc"), "WFP": w("Wfp"),
        "BQK": pp(bqkv[:2048]), "BO8": pp(inputs["bo"]), "BQ8": pp(inputs["bq"]),
        "BKC8": pp(bkv[:1024]), "BCO8": pp(inputs["bco"]),
        "BFC32": pp(inputs["bfc"]), "BFP8": pp(inputs["bfp"]),
        "LN1W": pp(inputs["ln1_w"]), "LN1B": pp(inputs["ln1_b"]),
        "LNCW": pp(inputs["lnc_w"]), "LNCB": pp(inputs["lnc_b"]),
        "LN2W": pp(inputs["ln2_w"]), "LN2B": pp(inputs["ln2_b"]),
        "T1M": t1, "M1M": m1, "E2": e2.astype(BF),
        "ONL8": np.ones((128, 8), np.float32),
        "ON8B": np.full((8, 128), 0.125, np.float32),
        "EPS8": np.full((8, 1), EPS, np.float32),
        "ONES16": np.ones((128, 16), np.float32).astype(BF),
        "KEEPBC": np.broadcast_to(keep, (128, TMY)).copy(),
        "VBBC": np.broadcast_to(bqkv[2048:], (128, C)).astype(BF).copy(),
        "VCBBC": np.broadcast_to(bkv[1024:], (128, C)).astype(BF).copy(),
        "PADS": pads, "PADC": padc,
    }


LAST_RESULTS = None


def kernel(**inputs):
    global LAST_RESULTS
    from concourse.bass_utils import run_bass_kernel_spmd

    if "nc" not in _CACHED:
        _CACHED["nc"] = _build_nc()
    nc = _CACHED["nc"]

    in_maps = [_host_prepare(core, inputs) for core in range(8)]
    trace = bool(int(os.environ.get("KERNEL_TRACE", "0")))
    res = run_bass_kernel_spmd(nc, in_maps, list(range(8)), trace=trace)
    LAST_RESULTS = res

    out = np.zeros((B, T, C), np.float32)
    for core in range(8):
        b, half = core // 2, core % 2
        chunks = [0, 3] if half == 0 else [1, 2]
        o = np.asarray(res.results[core]["OUT"])
        for j, c in enumerate(chunks):
            out[b, 256 * c:256 * (c + 1), :] = o[:, 256 * j:256 * (j + 1)].T
    return out

